# revision 68
# baseline (speedup 1.0000x reference)
"""Trainium2 Bass kernel for nn_CrossAttention (b=2, n=m=2048, dim=1024, 16 heads x 64).

Sharding: 8 cores = (batch b in {0,1}) x (head-group g in {0..3}, 4 heads each).
Per core: project q/k/v for its 4 heads (feature-major layouts), attention with
softmax (no max subtraction -- logits are bounded ~|3.3|), row sums via a ones
column appended to V, then an AllToAll over the 4 cores of each batch converts
head-sharding to row-sharding for the output MLP (relu(A@W1)@W2) + LayerNorm.

Schedule: the projection matmuls are software-pipelined into the attention
phase (which is Activation-engine bound on the softmax exp), and the
AllToAll is split into two 1MB halves by head-pair: the first fires after
the first two heads' attention and hides under the remaining attention; the
second overlaps the first half of the MLP's Y accumulation.
"""

import sys

if "/opt/trn_rl_repo" not in sys.path:
    sys.path.insert(0, "/opt/trn_rl_repo")

from contextlib import ExitStack

import numpy as np
import ml_dtypes

import concourse.bacc as bacc
import concourse.tile as tile
from concourse import mybir, library_config
from concourse.bass_utils import run_bass_kernel_spmd

DT = mybir.dt
BF16 = ml_dtypes.bfloat16

P = 128          # partitions
N = 2048         # tokens per batch
DIM = 1024       # model dim
HD = 64          # head dim
NH = 4           # heads per core
E = NH * HD      # 256 features per core
CT = DIM // P    # 8 contraction tiles
JT = N // P      # 16 key tiles
IBS = 512        # i-block size
IB = N // IBS    # 4 i-blocks
RQ = 512         # output rows per core
FT = DIM // P    # 8 f-tiles in MLP

_PROGRAM = None
LAST_RUN = None  # BassKernelResults of the most recent kernel() call


def build_program(sim_mode=False):
    nc = bacc.Bacc(None, num_devices=8)

    xT_d = nc.dram_tensor("xT", [DIM, N], DT.bfloat16, kind="ExternalInput")
    ctxT_d = nc.dram_tensor("ctxT", [DIM, N], DT.bfloat16, kind="ExternalInput")
    wq_d = nc.dram_tensor("wq", [DIM, E], DT.bfloat16, kind="ExternalInput")
    wk_d = nc.dram_tensor("wk", [DIM, E], DT.bfloat16, kind="ExternalInput")
    wv_d = nc.dram_tensor("wv", [DIM, E], DT.bfloat16, kind="ExternalInput")
    w1_d = nc.dram_tensor("w1", [DIM, DIM], DT.bfloat16, kind="ExternalInput")
    w2_d = nc.dram_tensor("w2", [DIM, DIM], DT.bfloat16, kind="ExternalInput")
    out_d = nc.dram_tensor("out", [RQ, DIM], DT.float32, kind="ExternalOutput")

    with tile.TileContext(nc) as tc:
        stack = ExitStack()
        with stack:
            nc.gpsimd.load_library(library_config.attnmlp)

            const = stack.enter_context(tc.tile_pool(name="const", bufs=1))
            eps_t = const.tile([P, 1], DT.float32, name="eps_t", tag="eps_t")
            nc.vector.memset(eps_t[:], 1e-5)

            # ---- persistent activation tiles ----
            qkv = stack.enter_context(tc.tile_pool(name="qkv", bufs=1))
            qT_t = [qkv.tile([P, N], DT.bfloat16, name=f"qT{i}", tag=f"qT{i}") for i in range(2)]
            kT_t = [qkv.tile([P, N], DT.bfloat16, name=f"kT{i}", tag=f"kT{i}") for i in range(2)]
            v_t = [qkv.tile([P, NH * 65], DT.bfloat16, name=f"v{j}", tag=f"v{j}") for j in range(JT)]
            aT_t = [qkv.tile([P, N], DT.bfloat16, name=f"aT{i}", tag=f"aT{i}") for i in range(2)]

            # ---- inputs: batched DMAs (one instr per tensor), issue order =
            # earliest-needed first. SBUF layout [P, c, cols]; the DRAM side
            # is rearranged so row 128c+p lands on partition p, block c.
            inp = stack.enter_context(tc.tile_pool(name="inputs", bufs=1))
            xT_s = inp.tile([P, CT, N], DT.bfloat16, name="xT", tag="xT")
            ctxT_s = inp.tile([P, CT, N], DT.bfloat16, name="cT", tag="cT")
            wq_s = inp.tile([P, CT, E], DT.bfloat16, name="wq", tag="wq")
            wk_s = inp.tile([P, CT, E], DT.bfloat16, name="wk", tag="wk")
            wv_s = inp.tile([P, CT, E], DT.bfloat16, name="wv", tag="wv")
            mlpw = stack.enter_context(tc.tile_pool(name="mlpw", bufs=1))
            w1_s = mlpw.tile([P, CT, DIM], DT.bfloat16, name="w1", tag="w1")
            w2_s = mlpw.tile([P, CT, DIM], DT.bfloat16, name="w2", tag="w2")
            xT_t = [xT_s[:, c, :] for c in range(CT)]
            ctxT_t = [ctxT_s[:, c, :] for c in range(CT)]
            wq_t = [wq_s[:, c, :] for c in range(CT)]
            wk_t = [wk_s[:, c, :] for c in range(CT)]
            wv_t = [wv_s[:, c, :] for c in range(CT)]
            w1_t = [w1_s[:, c, :] for c in range(CT)]
            w2_t = [w2_s[:, c, :] for c in range(CT)]

            def split_rows(d):  # [CT*P, cols] -> [P, CT, cols]
                return d[:].rearrange("(c p) n -> p c n", p=P)

            nc.sync.dma_start(wk_s[:], split_rows(wk_d))
            for c in range(CT):  # per-c so the kT psum groups chase arrivals
                nc.sync.dma_start(ctxT_s[:, c, :], ctxT_d[P * c:P * (c + 1), :])
            # only i-block 0 of x up front: unblocks the first dots
            nc.sync.dma_start(xT_s[:, :, 0:IBS], split_rows(xT_d)[:, :, 0:IBS])
            nc.sync.dma_start(wq_s[:], split_rows(wq_d))
            nc.sync.dma_start(wv_s[:], split_rows(wv_d))
            nc.sync.dma_start(xT_s[:, :, IBS:N], split_rows(xT_d)[:, :, IBS:N])
            nc.sync.dma_start(w1_s[:], split_rows(w1_d))
            nc.sync.dma_start(w2_s[:], split_rows(w2_d))

            # ---- a2a DRAM buffers, one pair per head-pair (pr) ----
            a2a_in = []
            a2a_out = []
            for pr in range(2):
                ain, ain_free = tc.tile([8 * P, RQ], DT.bfloat16, space="DRAM",
                                        name=f"a2a_in{pr}")
                aout, aout_free = tc.tile([8 * P, RQ], DT.bfloat16, space="DRAM",
                                          addr_space="Shared", name=f"a2a_out{pr}")
                stack.callback(ain_free)
                stack.callback(aout_free)
                a2a_in.append(ain)
                a2a_out.append(aout)

            # one projection psum group: {k,q}T[et] cols [IBS*blk, IBS*(blk+1)].
            # Emitted either whole, or as a list of per-c steps so the group
            # can be interleaved into the attention j-loop's PE slack.
            def proj_group_steps(kind, et, blk):
                wt, src, dst = ((wk_t, ctxT_t, kT_t) if kind == "k"
                                else (wq_t, xT_t, qT_t))
                state = {}

                def step(c):
                    def f():
                        if c == 0:
                            state["ps"] = pp.tile([P, IBS], DT.float32,
                                                  name="pps", tag="projps")
                        nc.tensor.matmul(
                            state["ps"][:], wt[c][:, P * et:P * (et + 1)],
                            src[c][:, IBS * blk:IBS * (blk + 1)],
                            start=(c == 0), stop=(c == CT - 1))
                    return f

                def copy():
                    nc.vector.tensor_copy(
                        dst[et][:, IBS * blk:IBS * (blk + 1)], state["ps"][:])

                return [step(c) for c in range(CT)] + [copy]

            def proj_kT(pp, et, blk):
                for f in proj_group_steps("k", et, blk):
                    f()

            def proj_qT(pp, et, ib):
                for f in proj_group_steps("q", et, ib):
                    f()

            def emit_a2a_chunk(pr, q, dup=True):
                # duplicate the group-chunk into both absolute halves: the
                # 8-core AllToAll delivers chunk s to core s; cores 0-3 read
                # the first half, 4-7 the second. With dup=False, write only
                # the half this core's batch peers read (pid branch) -- the
                # other half's slots carry stale bytes the other batch
                # ignores; used for the last chunk to shorten the chain into
                # the collective.
                src = aT_t[pr][:, RQ * q:RQ * (q + 1)]
                if dup:
                    nc.sync.dma_start(a2a_in[pr][P * q:P * (q + 1), :], src)
                    nc.sync.dma_start(a2a_in[pr][P * (q + 4):P * (q + 5), :], src)
                elif sim_mode:
                    nc.sync.dma_start(a2a_in[pr][P * q:P * (q + 1), :], src)
                else:
                    pid = nc.sync.partition_id()
                    with tc.If(pid < 4) as cmp:
                        nc.sync.dma_start(a2a_in[pr][P * q:P * (q + 1), :], src)
                    with cmp.Else():
                        nc.sync.dma_start(a2a_in[pr][P * (q + 4):P * (q + 5), :], src)

            def emit_a2a(pr):
                nc.gpsimd.collective_compute(
                    "AllToAll", mybir.AluOpType.bypass,
                    replica_groups=[[0, 1, 2, 3, 4, 5, 6, 7]],
                    ins=[a2a_in[pr].opt()], outs=[a2a_out[pr].opt()])

            # MLP activation tiles, allocated up front so the first half's
            # post-collective load can be issued mid-attention.
            # aTf_t[2g+pr] = A^T rows [256g+128pr : ...+128] = chunk from
            # same-batch peer g, head-pair pr
            mlp = stack.enter_context(tc.tile_pool(name="mlp", bufs=1))
            aTf_s = mlp.tile([P, CT, RQ], DT.bfloat16, name="aTf", tag="aTf")
            aTf_t = [aTf_s[:, c, :] for c in range(CT)]
            hT_t = [mlp.tile([P, RQ], DT.bfloat16, name=f"hT{c}", tag=f"hT{c}") for c in range(CT)]

            def load_aTf(pr, base):
                if pr == 0:
                    # one DMA: [P, g, RQ] <- 4 chunks (lands mid-attention)
                    dst = aTf_s.rearrange("p (g t) r -> p g t r", t=2)[:, :, pr, :]
                    src = a2a_out[pr][P * base:P * (base + 4), :]
                    nc.sync.dma_start(dst, src.rearrange("(g p) r -> p g r", p=P))
                else:
                    # per-chunk, in et order: the et-major Y-finish can start
                    # on chunk g0 while g1-3 stream in post-collective
                    for g in range(4):
                        nc.sync.dma_start(
                            aTf_s[:, 2 * g + 1, :],
                            a2a_out[pr][P * (base + g):P * (base + g + 1), :])

            def emit_load_aTf(pr):
                if sim_mode:
                    load_aTf(pr, 0)
                else:
                    pid = nc.sync.partition_id()
                    with tc.If(pid < 4) as cmp:
                        load_aTf(pr, 0)
                    with cmp.Else():
                        load_aTf(pr, 4)

            pp_ctx = ExitStack()
            pp = pp_ctx.enter_context(tc.tile_pool(name="pp", bufs=2, space="PSUM"))

            # one V projection tile: v_t[j] (+ its ones column)
            def proj_v(j):
                ps = pp.tile([P, E], DT.float32, name="vps", tag="projps")
                for c in range(CT):
                    nc.tensor.matmul(
                        ps[:], ctxT_t[c][:, P * j:P * (j + 1)], wv_t[c][:],
                        start=(c == 0), stop=(c == CT - 1))
                v_re = v_t[j].rearrange("p (h x) -> p h x", h=NH)
                nc.vector.tensor_copy(
                    v_re[:, :, 0:HD], ps.rearrange("p (h x) -> p h x", h=NH))
                nc.vector.memset(v_re[:, :, HD:65], 1.0)

            # ---- phase P0: kT (heads 0-1), qT (heads 0-1, i-block 0) ----
            # V is projected inside the first attention i-block's j-loop so
            # the Activation engine (softmax exp, the attention bottleneck)
            # starts ~14us earlier.
            for blk in range(4):
                proj_kT(pp, 0, blk)
            proj_qT(pp, 0, 0)

            # proj groups scattered into the attention phase's PE slack.
            # Constraint: qT(et,b) before i-block b of head-pair et; kT(1,*)
            # before head-pair 1. i-block 0 carries the V projection instead.
            scatter = {
                (0, 0): [("q", 0, 1)],  # late slots: needs the x-rest DMA
                (0, 1): [("q", 0, 2), ("k", 1, 0)],
                (0, 2): [("q", 0, 3), ("k", 1, 1)],
                (0, 3): [("k", 1, 2), ("k", 1, 3), ("q", 1, 0)],
                (1, 0): [("q", 1, 1)],
                (1, 1): [("q", 1, 2)],
                (1, 2): [("q", 1, 3)],
                (1, 3): [],
            }

            # ---- phase A: attention, two heads (one qT/kT tile) at a time ----
            with tc.tile_pool(name="s_ps", bufs=2, space="PSUM") as s_ps_pool, \
                 tc.tile_pool(name="acc_ps", bufs=1, space="PSUM") as acc_pool, \
                 tc.tile_pool(name="p_sb", bufs=4) as p_pool, \
                 tc.tile_pool(name="nrm", bufs=2) as nrm_pool:
                for pr in range(2):
                    for ib in range(IB):
                        isl = slice(IBS * ib, IBS * (ib + 1))
                        accs = [acc_pool.tile([P, IBS], DT.float32, name=f"acc{hh}", tag=f"acc{hh}")
                                for hh in range(2)]
                        # flatten this i-block's scattered proj groups into
                        # per-c steps, doled out across the j-loop (j>=2 so
                        # the dots->exp pipeline primes first)
                        steps = [f for g in scatter[(pr, ib)]
                                 for f in proj_group_steps(*g)]
                        # ib0's PE is saturated by V; push its group late
                        # (also after the x-rest DMA it depends on)
                        slots = (list(range(10, JT)) if (pr, ib) == (0, 0)
                                 else list(range(2, JT - 1)))
                        per_slot = -(-len(steps) // len(slots)) if steps else 0
                        # dots for j are emitted one iteration ahead of j's
                        # av so an av stall (waiting on exp) never delays
                        # the next dots on the in-order PE stream
                        def emit_dots(j):
                            sps = s_ps_pool.tile([P, 2 * IBS], DT.float32, name="sps", tag="sps")
                            for hh in range(2):
                                d = slice(HD * hh, HD * (hh + 1))
                                nc.tensor.matmul(
                                    sps[:, IBS * hh:IBS * (hh + 1)],
                                    kT_t[pr][d, P * j:P * (j + 1)], qT_t[pr][d, isl],
                                    start=True, stop=True)
                            pt = p_pool.tile([P, 2 * IBS], DT.bfloat16, name="pt", tag="pt")
                            nc.scalar.activation(pt[:], sps[:],
                                                 mybir.ActivationFunctionType.Exp,
                                                 scale=float(HD) ** -0.5)
                            return pt

                        def emit_av(j, pt):
                            for hh in range(2):
                                h = 2 * pr + hh
                                nc.tensor.matmul(
                                    accs[hh][0:65, :], v_t[j][:, 65 * h:65 * h + 65],
                                    pt[:, IBS * hh:IBS * (hh + 1)],
                                    start=(j == 0), stop=(j == JT - 1))

                        pts = {0: emit_dots(0)}
                        for j in range(JT):
                            if j + 1 < JT:
                                pts[j + 1] = emit_dots(j + 1)
                            # filler work between dots(j+1) and av(j): by the
                            # time the PE reaches av, exp(j) has retired, so
                            # the exp->av semaphore wait never stalls the PE
                            if pr == 0 and ib == 0:
                                proj_v(j)
                            if j in slots and steps:
                                k0 = slots.index(j) * per_slot
                                for f in steps[k0:k0 + per_slot]:
                                    f()
                            emit_av(j, pts.pop(j))
                        # copy accumulators to SBUF first: releases the acc
                        # psum ring ~1.5us earlier for the next i-block. The
                        # last i-block has no successor: normalize straight
                        # from PSUM to shorten the chain into collective #2.
                        if (pr, ib) == (1, IB - 1):
                            ans = [accs[hh][0:65, :] for hh in range(2)]
                        else:
                            ans = []
                            for hh in range(2):
                                an = nrm_pool.tile([65, IBS], DT.float32, name=f"an{hh}", tag=f"an{hh}")
                                nc.vector.tensor_copy(an[:], accs[hh][0:65, :])
                                ans.append(an)
                        rcps, bcs = [], []
                        for hh in range(2):
                            rcp = nrm_pool.tile([1, IBS], DT.float32, name="rcp", tag="rcp")
                            nc.vector.reciprocal(rcp[:], ans[hh][64:65, :])
                            rcps.append(rcp)
                        for hh in range(2):  # both Pool launches back-to-back
                            bc = nrm_pool.tile([HD, IBS], DT.float32, name="bc", tag="bc")
                            nc.gpsimd.partition_broadcast(bc[:], rcps[hh][:])
                            bcs.append(bc)
                        for hh in range(2):
                            nc.vector.tensor_tensor(
                                aT_t[pr][HD * hh:HD * (hh + 1), isl],
                                ans[hh][0:HD, :], bcs[hh][:], mybir.AluOpType.mult)
                        emit_a2a_chunk(pr, ib, dup=(pr, ib) != (1, IB - 1))
                        if ib == IB - 1:
                            emit_a2a(pr)  # fire this head-pair's exchange ASAP
                            if pr == 0:
                                emit_load_aTf(0)  # lands mid-attention
            pp_ctx.close()

            # ---- phase M: AllToAll results -> row-sharded MLP + LayerNorm ----
            emit_load_aTf(1)

            # Y^T = relu(W1^T A^T): accumulate the head-pair-0 features
            # (available after a2a#0) first so this runs under a2a#1.
            with tc.tile_pool(name="y_ps", bufs=1, space="PSUM") as y_ps_pool:
                yps = [y_ps_pool.tile([P, RQ], DT.float32, name=f"yps{ft}", tag=f"yps{ft}")
                       for ft in range(FT)]
                for ft in range(FT):
                    for i, et in enumerate([0, 2, 4, 6]):
                        nc.tensor.matmul(
                            yps[ft][:], w1_t[et][:, P * ft:P * (ft + 1)], aTf_t[et][:],
                            start=(i == 0), stop=False)
                for ft in range(FT):
                    for i, et in enumerate([1, 3, 5, 7]):
                        nc.tensor.matmul(
                            yps[ft][:], w1_t[et][:, P * ft:P * (ft + 1)], aTf_t[et][:],
                            start=False, stop=(i == 3))
                    nc.vector.tensor_scalar_max(hT_t[ft][:], yps[ft][:], 0.0)

            # Z = H W2 ; LayerNorm
            with tc.tile_pool(name="z_ps", bufs=3, space="PSUM") as z_ps_pool, \
                 tc.tile_pool(name="ln", bufs=3) as ln_pool:
                for it in range(RQ // P):
                    zps = z_ps_pool.tile([P, DIM], DT.float32, name="zps", tag="zps")
                    stats = ln_pool.tile([P, 2, 6], DT.float32, name="stats", tag="stats")
                    for gt in range(2):
                        for ft in range(FT):
                            nc.tensor.matmul(
                                zps[:, IBS * gt:IBS * (gt + 1)],
                                hT_t[ft][:, P * it:P * (it + 1)],
                                w2_t[ft][:, IBS * gt:IBS * (gt + 1)],
                                start=(ft == 0), stop=(ft == FT - 1))
                    for sg in range(2):
                        nc.vector.bn_stats(stats[:, sg, :], zps[:, IBS * sg:IBS * (sg + 1)])
                    mv = ln_pool.tile([P, 2], DT.float32, name="mv", tag="mv")
                    nc.vector.bn_aggr(mv[:], stats[:])
                    # mv[:,1] := 1/sqrt(var + eps)
                    nc.scalar.activation(mv[:, 1:2], mv[:, 1:2],
                                         mybir.ActivationFunctionType.Sqrt,
                                         bias=eps_t[:])
                    nc.vector.reciprocal(mv[:, 1:2], mv[:, 1:2])
                    # nb := -mean*rstd, then out = z*rstd + nb on the (idle)
                    # Activation engine. gamma is applied host-side.
                    nb = ln_pool.tile([P, 1], DT.float32, name="nb", tag="nb")
                    nc.vector.scalar_tensor_tensor(
                        out=nb[:], in0=mv[:, 0:1], scalar=-1.0,
                        op0=mybir.AluOpType.mult, in1=mv[:, 1:2],
                        op1=mybir.AluOpType.mult)
                    # normalize + store in column halves: the first half's
                    # store overlaps the second half's Activation pass
                    for sg in range(2):
                        csl = slice(IBS * sg, IBS * (sg + 1))
                        ot = ln_pool.tile([P, IBS], DT.float32, name=f"ot{sg}", tag=f"ot{sg}")
                        nc.scalar.activation(ot[:], zps[:, csl],
                                             mybir.ActivationFunctionType.Identity,
                                             scale=mv[:, 1:2], bias=nb[:])
                        nc.sync.dma_start(out_d[P * it:P * (it + 1), csl], ot[:])

    nc.finalize()
    return nc


def _get_program():
    global _PROGRAM
    if _PROGRAM is None:
        _PROGRAM = build_program()
    return _PROGRAM


def prepare_in_maps(x, context, w_kv, w_q, w_out1, w_out2):
    x = np.asarray(x, np.float32)
    context = np.asarray(context, np.float32)
    w_kv = np.asarray(w_kv, np.float32)
    w_q = np.asarray(w_q, np.float32)
    w1 = np.ascontiguousarray(np.asarray(w_out1, np.float32).astype(BF16))
    w2 = np.ascontiguousarray(np.asarray(w_out2, np.float32).astype(BF16))
    xT = [np.ascontiguousarray(x[b].T.astype(BF16)) for b in range(2)]
    ctxT = [np.ascontiguousarray(context[b].T.astype(BF16)) for b in range(2)]
    in_maps = []
    for c in range(8):
        b, g = divmod(c, 4)
        e0 = E * g
        in_maps.append({
            "xT": xT[b],
            "ctxT": ctxT[b],
            "wq": np.ascontiguousarray(w_q[:, e0:e0 + E].astype(BF16)),
            "wk": np.ascontiguousarray(w_kv[:, e0:e0 + E].astype(BF16)),
            "wv": np.ascontiguousarray(w_kv[:, DIM + e0:DIM + e0 + E].astype(BF16)),
            "w1": w1,
            "w2": w2,
        })
    return in_maps


def assemble_output(per_core_outs):
    out = np.empty((2, N, DIM), np.float32)
    for c in range(8):
        b, g = divmod(c, 4)
        out[b, RQ * g:RQ * (g + 1), :] = per_core_outs[c]
    return out


def kernel(x, context, w_kv, w_q, w_out1, w_out2, gamma):
    global LAST_RUN
    in_maps = prepare_in_maps(x, context, w_kv, w_q, w_out1, w_out2)
    nc = _get_program()
    res = run_bass_kernel_spmd(nc, in_maps, list(range(8)))
    LAST_RUN = res
    out = assemble_output([res.results[c]["out"] for c in range(8)])
    # LayerNorm's gamma multiplies the final output elementwise -- applied
    # host-side, exact for any gamma.
    return out * np.asarray(gamma, np.float32).reshape(1, 1, DIM)


# revision 69
# speedup vs baseline: 1.0024x; 1.0024x over previous
"""Trainium2 Bass kernel for nn_CrossAttention (b=2, n=m=2048, dim=1024, 16 heads x 64).

Sharding: 8 cores = (batch b in {0,1}) x (head-group g in {0..3}, 4 heads each).
Per core: project q/k/v for its 4 heads (feature-major layouts), attention with
softmax (no max subtraction -- logits are bounded ~|3.3|), row sums via a ones
column appended to V, then an AllToAll over the 4 cores of each batch converts
head-sharding to row-sharding for the output MLP (relu(A@W1)@W2) + LayerNorm.

Schedule: the projection matmuls are software-pipelined into the attention
phase (which is Activation-engine bound on the softmax exp), and the
AllToAll is split into two 1MB halves by head-pair: the first fires after
the first two heads' attention and hides under the remaining attention; the
second overlaps the first half of the MLP's Y accumulation.
"""

import sys

if "/opt/trn_rl_repo" not in sys.path:
    sys.path.insert(0, "/opt/trn_rl_repo")

from contextlib import ExitStack

import numpy as np
import ml_dtypes

import concourse.bacc as bacc
import concourse.tile as tile
from concourse import mybir, library_config
from concourse.bass_utils import run_bass_kernel_spmd

DT = mybir.dt
BF16 = ml_dtypes.bfloat16

P = 128          # partitions
N = 2048         # tokens per batch
DIM = 1024       # model dim
HD = 64          # head dim
NH = 4           # heads per core
E = NH * HD      # 256 features per core
CT = DIM // P    # 8 contraction tiles
JT = N // P      # 16 key tiles
IBS = 512        # i-block size
IB = N // IBS    # 4 i-blocks
RQ = 512         # output rows per core
FT = DIM // P    # 8 f-tiles in MLP

_PROGRAM = None
LAST_RUN = None  # BassKernelResults of the most recent kernel() call


def build_program(sim_mode=False):
    nc = bacc.Bacc(None, num_devices=8)

    xT_d = nc.dram_tensor("xT", [DIM, N], DT.bfloat16, kind="ExternalInput")
    ctxT_d = nc.dram_tensor("ctxT", [DIM, N], DT.bfloat16, kind="ExternalInput")
    wq_d = nc.dram_tensor("wq", [DIM, E], DT.bfloat16, kind="ExternalInput")
    wk_d = nc.dram_tensor("wk", [DIM, E], DT.bfloat16, kind="ExternalInput")
    wv_d = nc.dram_tensor("wv", [DIM, E], DT.bfloat16, kind="ExternalInput")
    w1_d = nc.dram_tensor("w1", [DIM, DIM], DT.bfloat16, kind="ExternalInput")
    w2_d = nc.dram_tensor("w2", [DIM, DIM], DT.bfloat16, kind="ExternalInput")
    out_d = nc.dram_tensor("out", [RQ, DIM], DT.float32, kind="ExternalOutput")

    with tile.TileContext(nc) as tc:
        stack = ExitStack()
        with stack:
            nc.gpsimd.load_library(library_config.attnmlp)

            const = stack.enter_context(tc.tile_pool(name="const", bufs=1))
            eps_t = const.tile([P, 1], DT.float32, name="eps_t", tag="eps_t")
            nc.vector.memset(eps_t[:], 1e-5)

            # ---- persistent activation tiles ----
            qkv = stack.enter_context(tc.tile_pool(name="qkv", bufs=1))
            qT_t = [qkv.tile([P, N], DT.bfloat16, name=f"qT{i}", tag=f"qT{i}") for i in range(2)]
            kT_t = [qkv.tile([P, N], DT.bfloat16, name=f"kT{i}", tag=f"kT{i}") for i in range(2)]
            v_t = [qkv.tile([P, NH * 65], DT.bfloat16, name=f"v{j}", tag=f"v{j}") for j in range(JT)]
            aT_t = [qkv.tile([P, N], DT.bfloat16, name=f"aT{i}", tag=f"aT{i}") for i in range(2)]

            # ---- inputs: batched DMAs (one instr per tensor), issue order =
            # earliest-needed first. SBUF layout [P, c, cols]; the DRAM side
            # is rearranged so row 128c+p lands on partition p, block c.
            inp = stack.enter_context(tc.tile_pool(name="inputs", bufs=1))
            xT_s = inp.tile([P, CT, N], DT.bfloat16, name="xT", tag="xT")
            ctxT_s = inp.tile([P, CT, N], DT.bfloat16, name="cT", tag="cT")
            wq_s = inp.tile([P, CT, E], DT.bfloat16, name="wq", tag="wq")
            wk_s = inp.tile([P, CT, E], DT.bfloat16, name="wk", tag="wk")
            wv_s = inp.tile([P, CT, E], DT.bfloat16, name="wv", tag="wv")
            mlpw = stack.enter_context(tc.tile_pool(name="mlpw", bufs=1))
            w1_s = mlpw.tile([P, CT, DIM], DT.bfloat16, name="w1", tag="w1")
            w2_s = mlpw.tile([P, CT, DIM], DT.bfloat16, name="w2", tag="w2")
            xT_t = [xT_s[:, c, :] for c in range(CT)]
            ctxT_t = [ctxT_s[:, c, :] for c in range(CT)]
            wq_t = [wq_s[:, c, :] for c in range(CT)]
            wk_t = [wk_s[:, c, :] for c in range(CT)]
            wv_t = [wv_s[:, c, :] for c in range(CT)]
            w1_t = [w1_s[:, c, :] for c in range(CT)]
            w2_t = [w2_s[:, c, :] for c in range(CT)]

            def split_rows(d):  # [CT*P, cols] -> [P, CT, cols]
                return d[:].rearrange("(c p) n -> p c n", p=P)

            nc.sync.dma_start(wk_s[:], split_rows(wk_d))
            for c in range(CT):  # per-c so the kT psum groups chase arrivals
                nc.sync.dma_start(ctxT_s[:, c, :], ctxT_d[P * c:P * (c + 1), :])
            # only i-block 0 of x up front: unblocks the first dots
            nc.sync.dma_start(xT_s[:, :, 0:IBS], split_rows(xT_d)[:, :, 0:IBS])
            nc.sync.dma_start(wq_s[:], split_rows(wq_d))
            nc.sync.dma_start(wv_s[:], split_rows(wv_d))
            nc.sync.dma_start(xT_s[:, :, IBS:N], split_rows(xT_d)[:, :, IBS:N])
            nc.sync.dma_start(w1_s[:], split_rows(w1_d))
            nc.sync.dma_start(w2_s[:], split_rows(w2_d))

            # ---- a2a DRAM buffers, one pair per head-pair (pr) ----
            a2a_in = []
            a2a_out = []
            for pr in range(2):
                ain, ain_free = tc.tile([8 * P, RQ], DT.bfloat16, space="DRAM",
                                        name=f"a2a_in{pr}")
                aout, aout_free = tc.tile([8 * P, RQ], DT.bfloat16, space="DRAM",
                                          addr_space="Shared", name=f"a2a_out{pr}")
                stack.callback(ain_free)
                stack.callback(aout_free)
                a2a_in.append(ain)
                a2a_out.append(aout)

            # one projection psum group: {k,q}T[et] cols [IBS*blk, IBS*(blk+1)].
            # Emitted either whole, or as a list of per-c steps so the group
            # can be interleaved into the attention j-loop's PE slack.
            def proj_group_steps(kind, et, blk):
                wt, src, dst = ((wk_t, ctxT_t, kT_t) if kind == "k"
                                else (wq_t, xT_t, qT_t))
                state = {}

                def step(c):
                    def f():
                        if c == 0:
                            state["ps"] = pp.tile([P, IBS], DT.float32,
                                                  name="pps", tag="projps")
                        nc.tensor.matmul(
                            state["ps"][:], wt[c][:, P * et:P * (et + 1)],
                            src[c][:, IBS * blk:IBS * (blk + 1)],
                            start=(c == 0), stop=(c == CT - 1))
                    return f

                def copy():
                    nc.vector.tensor_copy(
                        dst[et][:, IBS * blk:IBS * (blk + 1)], state["ps"][:])

                return [step(c) for c in range(CT)] + [copy]

            def proj_kT(pp, et, blk):
                for f in proj_group_steps("k", et, blk):
                    f()

            def proj_qT(pp, et, ib):
                for f in proj_group_steps("q", et, ib):
                    f()

            def emit_a2a_chunk(pr, q, dup=True):
                # duplicate the group-chunk into both absolute halves: the
                # 8-core AllToAll delivers chunk s to core s; cores 0-3 read
                # the first half, 4-7 the second. With dup=False, write only
                # the half this core's batch peers read (pid branch) -- the
                # other half's slots carry stale bytes the other batch
                # ignores; used for the last chunk to shorten the chain into
                # the collective.
                src = aT_t[pr][:, RQ * q:RQ * (q + 1)]
                if dup:
                    nc.sync.dma_start(a2a_in[pr][P * q:P * (q + 1), :], src)
                    nc.sync.dma_start(a2a_in[pr][P * (q + 4):P * (q + 5), :], src)
                elif sim_mode:
                    nc.sync.dma_start(a2a_in[pr][P * q:P * (q + 1), :], src)
                else:
                    pid = nc.sync.partition_id()
                    with tc.If(pid < 4) as cmp:
                        nc.sync.dma_start(a2a_in[pr][P * q:P * (q + 1), :], src)
                    with cmp.Else():
                        nc.sync.dma_start(a2a_in[pr][P * (q + 4):P * (q + 5), :], src)

            def emit_a2a(pr):
                nc.gpsimd.collective_compute(
                    "AllToAll", mybir.AluOpType.bypass,
                    replica_groups=[[0, 1, 2, 3, 4, 5, 6, 7]],
                    ins=[a2a_in[pr].opt()], outs=[a2a_out[pr].opt()])

            # MLP activation tiles, allocated up front so the first half's
            # post-collective load can be issued mid-attention.
            # aTf_t[2g+pr] = A^T rows [256g+128pr : ...+128] = chunk from
            # same-batch peer g, head-pair pr
            mlp = stack.enter_context(tc.tile_pool(name="mlp", bufs=1))
            aTf_s = mlp.tile([P, CT, RQ], DT.bfloat16, name="aTf", tag="aTf")
            aTf_t = [aTf_s[:, c, :] for c in range(CT)]
            hT_t = [mlp.tile([P, RQ], DT.bfloat16, name=f"hT{c}", tag=f"hT{c}") for c in range(CT)]

            def load_aTf(pr, base):
                if pr == 0:
                    # one DMA: [P, g, RQ] <- 4 chunks (lands mid-attention)
                    dst = aTf_s.rearrange("p (g t) r -> p g t r", t=2)[:, :, pr, :]
                    src = a2a_out[pr][P * base:P * (base + 4), :]
                    nc.sync.dma_start(dst, src.rearrange("(g p) r -> p g r", p=P))
                else:
                    # per-chunk, in et order: the et-major Y-finish can start
                    # on chunk g0 while g1-3 stream in post-collective
                    for g in range(4):
                        nc.sync.dma_start(
                            aTf_s[:, 2 * g + 1, :],
                            a2a_out[pr][P * (base + g):P * (base + g + 1), :])

            def emit_load_aTf(pr):
                if sim_mode:
                    load_aTf(pr, 0)
                else:
                    pid = nc.sync.partition_id()
                    with tc.If(pid < 4) as cmp:
                        load_aTf(pr, 0)
                    with cmp.Else():
                        load_aTf(pr, 4)

            pp_ctx = ExitStack()
            pp = pp_ctx.enter_context(tc.tile_pool(name="pp", bufs=2, space="PSUM"))

            # one V projection tile: v_t[j] (+ its ones column)
            def proj_v(j):
                ps = pp.tile([P, E], DT.float32, name="vps", tag="projps")
                for c in range(CT):
                    nc.tensor.matmul(
                        ps[:], ctxT_t[c][:, P * j:P * (j + 1)], wv_t[c][:],
                        start=(c == 0), stop=(c == CT - 1))
                v_re = v_t[j].rearrange("p (h x) -> p h x", h=NH)
                nc.vector.tensor_copy(
                    v_re[:, :, 0:HD], ps.rearrange("p (h x) -> p h x", h=NH))
                nc.vector.memset(v_re[:, :, HD:65], 1.0)

            # ---- phase P0: kT (heads 0-1), qT (heads 0-1, i-block 0) ----
            # V is projected inside the first attention i-block's j-loop so
            # the Activation engine (softmax exp, the attention bottleneck)
            # starts ~14us earlier.
            for blk in range(4):
                proj_kT(pp, 0, blk)
            proj_qT(pp, 0, 0)

            # proj groups scattered into the attention phase's PE slack.
            # Constraint: qT(et,b) before i-block b of head-pair et; kT(1,*)
            # before head-pair 1. i-block 0 carries the V projection instead.
            scatter = {
                (0, 0): [("q", 0, 1)],  # late slots: needs the x-rest DMA
                (0, 1): [("q", 0, 2), ("k", 1, 0)],
                (0, 2): [("q", 0, 3), ("k", 1, 1)],
                (0, 3): [("k", 1, 2), ("k", 1, 3), ("q", 1, 0)],
                (1, 0): [("q", 1, 1)],
                (1, 1): [("q", 1, 2)],
                (1, 2): [("q", 1, 3)],
                (1, 3): [],
            }

            # ---- phase A: attention, two heads (one qT/kT tile) at a time ----
            with tc.tile_pool(name="s_ps", bufs=2, space="PSUM") as s_ps_pool, \
                 tc.tile_pool(name="acc_ps", bufs=1, space="PSUM") as acc_pool, \
                 tc.tile_pool(name="p_sb", bufs=4) as p_pool, \
                 tc.tile_pool(name="nrm", bufs=2) as nrm_pool:
                for pr in range(2):
                    for ib in range(IB):
                        isl = slice(IBS * ib, IBS * (ib + 1))
                        accs = [acc_pool.tile([P, IBS], DT.float32, name=f"acc{hh}", tag=f"acc{hh}")
                                for hh in range(2)]
                        # flatten this i-block's scattered proj groups into
                        # per-c steps, doled out across the j-loop (j>=2 so
                        # the dots->exp pipeline primes first)
                        steps = [f for g in scatter[(pr, ib)]
                                 for f in proj_group_steps(*g)]
                        # ib0's PE is saturated by V; push its group late
                        # (also after the x-rest DMA it depends on)
                        slots = (list(range(10, JT)) if (pr, ib) == (0, 0)
                                 else list(range(1, JT - 1)))
                        per_slot = -(-len(steps) // len(slots)) if steps else 0
                        # dots for j are emitted one iteration ahead of j's
                        # av so an av stall (waiting on exp) never delays
                        # the next dots on the in-order PE stream
                        def emit_dots(j):
                            sps = s_ps_pool.tile([P, 2 * IBS], DT.float32, name="sps", tag="sps")
                            for hh in range(2):
                                d = slice(HD * hh, HD * (hh + 1))
                                nc.tensor.matmul(
                                    sps[:, IBS * hh:IBS * (hh + 1)],
                                    kT_t[pr][d, P * j:P * (j + 1)], qT_t[pr][d, isl],
                                    start=True, stop=True)
                            pt = p_pool.tile([P, 2 * IBS], DT.bfloat16, name="pt", tag="pt")
                            nc.scalar.activation(pt[:], sps[:],
                                                 mybir.ActivationFunctionType.Exp,
                                                 scale=float(HD) ** -0.5)
                            return pt

                        def emit_av(j, pt):
                            for hh in range(2):
                                h = 2 * pr + hh
                                nc.tensor.matmul(
                                    accs[hh][0:65, :], v_t[j][:, 65 * h:65 * h + 65],
                                    pt[:, IBS * hh:IBS * (hh + 1)],
                                    start=(j == 0), stop=(j == JT - 1))

                        pts = {0: emit_dots(0)}
                        for j in range(JT):
                            if j + 1 < JT:
                                pts[j + 1] = emit_dots(j + 1)
                            # filler work between dots(j+1) and av(j): by the
                            # time the PE reaches av, exp(j) has retired, so
                            # the exp->av semaphore wait never stalls the PE
                            if pr == 0 and ib == 0:
                                proj_v(j)
                            if j in slots and steps:
                                k0 = slots.index(j) * per_slot
                                for f in steps[k0:k0 + per_slot]:
                                    f()
                            emit_av(j, pts.pop(j))
                        # copy accumulators to SBUF first: releases the acc
                        # psum ring ~1.5us earlier for the next i-block. The
                        # last i-block has no successor: normalize straight
                        # from PSUM to shorten the chain into collective #2.
                        if (pr, ib) == (1, IB - 1):
                            ans = [accs[hh][0:65, :] for hh in range(2)]
                        else:
                            ans = []
                            for hh in range(2):
                                an = nrm_pool.tile([65, IBS], DT.float32, name=f"an{hh}", tag=f"an{hh}")
                                nc.vector.tensor_copy(an[:], accs[hh][0:65, :])
                                ans.append(an)
                        rcps, bcs = [], []
                        for hh in range(2):
                            rcp = nrm_pool.tile([1, IBS], DT.float32, name="rcp", tag="rcp")
                            nc.vector.reciprocal(rcp[:], ans[hh][64:65, :])
                            rcps.append(rcp)
                        for hh in range(2):  # both Pool launches back-to-back
                            bc = nrm_pool.tile([HD, IBS], DT.float32, name="bc", tag="bc")
                            nc.gpsimd.partition_broadcast(bc[:], rcps[hh][:])
                            bcs.append(bc)
                        for hh in range(2):
                            nc.vector.tensor_tensor(
                                aT_t[pr][HD * hh:HD * (hh + 1), isl],
                                ans[hh][0:HD, :], bcs[hh][:], mybir.AluOpType.mult)
                        emit_a2a_chunk(pr, ib, dup=(pr, ib) != (1, IB - 1))
                        if ib == IB - 1:
                            emit_a2a(pr)  # fire this head-pair's exchange ASAP
                            if pr == 0:
                                emit_load_aTf(0)  # lands mid-attention
            pp_ctx.close()

            # ---- phase M: AllToAll results -> row-sharded MLP + LayerNorm ----
            emit_load_aTf(1)

            # Y^T = relu(W1^T A^T): accumulate the head-pair-0 features
            # (available after a2a#0) first so this runs under a2a#1.
            with tc.tile_pool(name="y_ps", bufs=1, space="PSUM") as y_ps_pool:
                yps = [y_ps_pool.tile([P, RQ], DT.float32, name=f"yps{ft}", tag=f"yps{ft}")
                       for ft in range(FT)]
                for ft in range(FT):
                    for i, et in enumerate([0, 2, 4, 6]):
                        nc.tensor.matmul(
                            yps[ft][:], w1_t[et][:, P * ft:P * (ft + 1)], aTf_t[et][:],
                            start=(i == 0), stop=False)
                for ft in range(FT):
                    for i, et in enumerate([1, 3, 5, 7]):
                        nc.tensor.matmul(
                            yps[ft][:], w1_t[et][:, P * ft:P * (ft + 1)], aTf_t[et][:],
                            start=False, stop=(i == 3))
                    nc.vector.tensor_scalar_max(hT_t[ft][:], yps[ft][:], 0.0)

            # Z = H W2 ; LayerNorm
            with tc.tile_pool(name="z_ps", bufs=3, space="PSUM") as z_ps_pool, \
                 tc.tile_pool(name="ln", bufs=3) as ln_pool:
                for it in range(RQ // P):
                    zps = z_ps_pool.tile([P, DIM], DT.float32, name="zps", tag="zps")
                    stats = ln_pool.tile([P, 2, 6], DT.float32, name="stats", tag="stats")
                    for gt in range(2):
                        for ft in range(FT):
                            nc.tensor.matmul(
                                zps[:, IBS * gt:IBS * (gt + 1)],
                                hT_t[ft][:, P * it:P * (it + 1)],
                                w2_t[ft][:, IBS * gt:IBS * (gt + 1)],
                                start=(ft == 0), stop=(ft == FT - 1))
                    for sg in range(2):
                        nc.vector.bn_stats(stats[:, sg, :], zps[:, IBS * sg:IBS * (sg + 1)])
                    mv = ln_pool.tile([P, 2], DT.float32, name="mv", tag="mv")
                    nc.vector.bn_aggr(mv[:], stats[:])
                    # mv[:,1] := 1/sqrt(var + eps)
                    nc.scalar.activation(mv[:, 1:2], mv[:, 1:2],
                                         mybir.ActivationFunctionType.Sqrt,
                                         bias=eps_t[:])
                    nc.vector.reciprocal(mv[:, 1:2], mv[:, 1:2])
                    # nb := -mean*rstd, then out = z*rstd + nb on the (idle)
                    # Activation engine. gamma is applied host-side.
                    nb = ln_pool.tile([P, 1], DT.float32, name="nb", tag="nb")
                    nc.vector.scalar_tensor_tensor(
                        out=nb[:], in0=mv[:, 0:1], scalar=-1.0,
                        op0=mybir.AluOpType.mult, in1=mv[:, 1:2],
                        op1=mybir.AluOpType.mult)
                    # normalize + store in column halves: the first half's
                    # store overlaps the second half's Activation pass
                    for sg in range(2):
                        csl = slice(IBS * sg, IBS * (sg + 1))
                        ot = ln_pool.tile([P, IBS], DT.float32, name=f"ot{sg}", tag=f"ot{sg}")
                        nc.scalar.activation(ot[:], zps[:, csl],
                                             mybir.ActivationFunctionType.Identity,
                                             scale=mv[:, 1:2], bias=nb[:])
                        nc.sync.dma_start(out_d[P * it:P * (it + 1), csl], ot[:])

    nc.finalize()
    return nc


def _get_program():
    global _PROGRAM
    if _PROGRAM is None:
        _PROGRAM = build_program()
    return _PROGRAM


def prepare_in_maps(x, context, w_kv, w_q, w_out1, w_out2):
    x = np.asarray(x, np.float32)
    context = np.asarray(context, np.float32)
    w_kv = np.asarray(w_kv, np.float32)
    w_q = np.asarray(w_q, np.float32)
    w1 = np.ascontiguousarray(np.asarray(w_out1, np.float32).astype(BF16))
    w2 = np.ascontiguousarray(np.asarray(w_out2, np.float32).astype(BF16))
    xT = [np.ascontiguousarray(x[b].T.astype(BF16)) for b in range(2)]
    ctxT = [np.ascontiguousarray(context[b].T.astype(BF16)) for b in range(2)]
    in_maps = []
    for c in range(8):
        b, g = divmod(c, 4)
        e0 = E * g
        in_maps.append({
            "xT": xT[b],
            "ctxT": ctxT[b],
            "wq": np.ascontiguousarray(w_q[:, e0:e0 + E].astype(BF16)),
            "wk": np.ascontiguousarray(w_kv[:, e0:e0 + E].astype(BF16)),
            "wv": np.ascontiguousarray(w_kv[:, DIM + e0:DIM + e0 + E].astype(BF16)),
            "w1": w1,
            "w2": w2,
        })
    return in_maps


def assemble_output(per_core_outs):
    out = np.empty((2, N, DIM), np.float32)
    for c in range(8):
        b, g = divmod(c, 4)
        out[b, RQ * g:RQ * (g + 1), :] = per_core_outs[c]
    return out


def kernel(x, context, w_kv, w_q, w_out1, w_out2, gamma):
    global LAST_RUN
    in_maps = prepare_in_maps(x, context, w_kv, w_q, w_out1, w_out2)
    nc = _get_program()
    res = run_bass_kernel_spmd(nc, in_maps, list(range(8)))
    LAST_RUN = res
    out = assemble_output([res.results[c]["out"] for c in range(8)])
    # LayerNorm's gamma multiplies the final output elementwise -- applied
    # host-side, exact for any gamma.
    return out * np.asarray(gamma, np.float32).reshape(1, 1, DIM)
